# revision 22
# baseline (speedup 1.0000x reference)
"""Bass/Trainium2 kernel for nn_BayesianSkipgram (KL loss over skip-gram posterior).

Strategy (8 NeuronCores, data-parallel over batch; Bs=1024 per core):
  - Two-level gather, fully on-chip staging:
      stage 1: 4 bucket-compacted gathers (int16 local ids per 32767-row
               vocab bucket) land embedding rows for ALL 11264 token
               instances (ctx+x, no dedup) in an SBUF staging tile;
               4 more calls stage the fp16 prior rows [U_b-m0 | s0].
      stage 2: SBUF-source transpose-mode dma_gather (idx = staged slot id
               with tokens_per_rank=128) lands [E, token] tiles directly in
               original (b, c) order -- no HBM staging round trip, no PE
               transposes. 6 calls total (x, priors, 4 ctx chunks).
  - Projection RcT[D, tok] = M_w @ embT via PE (bf16) per 512-col PSUM bank,
    relu+bias via ACT, context sum via strided free-axis reduce.
  - KL computed in TRANSPOSED orientation [D, b]: mu/z via 2 matmuls each
    (uwt/wwt halves vs h1/h2), -(m0 - U_b) accumulated into the mu PSUM via
    an identity-f16 matmul, W_b folded in as an ACT bias.
  - 1/sigma = 1/softplus(z) as a degree-2 polynomial in z (|z| < 0.07 at
    this model scale; 8e-5 rel err over 2x the range); both log terms come
    from one ACT ln: ln sigma - ln s0 = -ln(s0/sigma) = -ln(s0 * rs).
  - Final sum over D via a ones-vector fp32 matmul (partition reduce on PE),
    kl = 0.5*sum - D/2 via ACT scale+bias; output is [1, Bs] f32.
  - Work is pipelined per ctx chunk: KL for batch half 0 runs while chunks
    2-3 are still gathering.
Host work is sharding/layout only: dtype casts, bucket sorting and index
packing, weight transposition, output reassembly.
"""

import numpy as np
import ml_dtypes

import concourse.bass as bass
import concourse.mybir as mybir
from concourse import bacc
from concourse import tile
from concourse.bass_utils import run_bass_kernel_spmd
from concourse.library_config import mlp

# Problem constants (hardcoded per harness contract)
V, E, D, B, C = 100000, 256, 128, 8192, 10
NCORES = 8
Bs = B // NCORES            # 1024 batch items per core
P = 128
NTOK = Bs * C + Bs          # 11264 gathered token instances (ctx then x)
BK = 32767                  # int16 vocab bucket size
NBK = 4
ECAPS = (3712, 3712, 3712, 384)      # emb stage-1 per-bucket caps (dedup'd)
EBASE = (0, 3712, 7424, 11136)
ESLOTS = sum(ECAPS)                  # 11520 staging slots
PCAPS = (384, 384, 384, 128)         # prior stage-1 per-bucket caps
PBASE = (0, 384, 768, 1152)
PSLOTS = sum(PCAPS)                  # 1280
S1W = 1024                           # stage-1 window cap (ucode limit)
S2W = 512                            # transpose-mode window cap (ucode limit)
NCH = 4
TPC = (Bs * C) // NCH                # 2560 ctx tokens per stage-2 chunk
HB = Bs // 2                         # 512-wide KL half

# 1/softplus(z) ~= RC0 + RC1*z + RC2*z^2 (fit on |z| <= 0.125)
RC0, RC1, RC2 = 1.44268652, -1.04204494, 0.49387287

F32 = mybir.dt.float32
BF16 = mybir.dt.bfloat16
F16 = mybir.dt.float16
F8 = mybir.dt.float8e4
I32 = mybir.dt.int32
I16 = mybir.dt.int16

_CACHE = {}
last_results = None  # set by kernel(); test.py reads exec_time_ns from here


def _build_nc():
    nc = bacc.Bacc(
        "TRN2",
        target_bir_lowering=False,
        debug=False,
        num_devices=NCORES,
        num_swdge_queues=4,
    )

    emb = nc.dram_tensor("emb", [V, E], F8, kind="ExternalInput")
    pcat = nc.dram_tensor("pcat", [V, 2 * D], F16, kind="ExternalInput")
    pk16 = nc.dram_tensor("pk16", [P, ESLOTS // 16 + Bs * C // 16 + Bs // 16
                                   + PSLOTS // 16 + Bs // 16], I16,
                          kind="ExternalInput")
    pkw = nc.dram_tensor("pkw", [P, 2 * 2 * D], BF16, kind="ExternalInput")
    pk8 = nc.dram_tensor("pk8", [P, 2 * D], F8, kind="ExternalInput")
    pkh = nc.dram_tensor("pkh", [P, P], F16, kind="ExternalInput")
    pkf = nc.dram_tensor("pkf", [P, 4], F32, kind="ExternalInput")
    klo = nc.dram_tensor("klo", [1, Bs], F32, kind="ExternalOutput")
    # HBM staging (ExternalOutput => contiguous runtime-allocated tensors;
    # Internal DRAM scratch may be paged, breaking flat base+idx*stride)
    staged = nc.dram_tensor("staged", [ESLOTS, E], F8, kind="ExternalOutput")
    staged_pr = nc.dram_tensor("staged_pr", [PSLOTS, 2 * D], F16,
                               kind="ExternalOutput")

    Relu = mybir.ActivationFunctionType.Relu
    Identity = mybir.ActivationFunctionType.Identity
    Ln = mybir.ActivationFunctionType.Ln
    TS = mybir.AluOpType
    AX = mybir.AxisListType.X

    # pk16 column offsets (int16 units)
    O_SIDX = 0
    O_RIDX = O_SIDX + ESLOTS // 16           # ctx stage-2 slots
    O_XIDX = O_RIDX + Bs * C // 16           # x stage-2 slots
    O_PIDX = O_XIDX + Bs // 16               # prior stage-1 local ids
    O_RPIDX = O_PIDX + PSLOTS // 16          # prior stage-2 slots

    def nextq():
        # placeholder; real queue assignment happens post-schedule, derived
        # from the Tile-assigned DMASW sem lane (one lane must map to exactly
        # one SWDGE queue)
        return 0

    with tile.TileContext(nc) as tc:
        with (
            tc.tile_pool(name="const", bufs=1) as const,
            tc.tile_pool(name="pers", bufs=1) as pers,
            tc.tile_pool(name="wtp", bufs=8) as wtp,
            tc.tile_pool(name="psp", bufs=3, space="PSUM") as psp,
            tc.tile_pool(name="psm", bufs=2, space="PSUM") as psm,
            tc.tile_pool(name="psr", bufs=2, space="PSUM") as psr,
        ):
            nc.gpsimd.load_library(mlp)

            # ---- constants into SBUF (5 DMAs) ----
            pk16_s = const.tile([P, pk16.shape[1]], I16)
            nc.sync.dma_start(out=pk16_s[:], in_=pk16[:])
            pkw_s = const.tile([P, 2 * 2 * D], BF16)
            nc.sync.dma_start(out=pkw_s[:], in_=pkw[:])
            mw8_s = const.tile([P, 2 * D], F8)
            nc.sync.dma_start(out=mw8_s[:], in_=pk8[:])
            ident_s = const.tile([P, P], F16)
            nc.sync.dma_start(out=ident_s[:], in_=pkh[:])
            pkf_s = const.tile([P, 4], F32)
            nc.sync.dma_start(out=pkf_s[:], in_=pkf[:])

            uwt_s = pkw_s[:, 0:2 * D]
            wwt_s = pkw_s[:, 2 * D:4 * D]
            wbT = pkf_s[:, 0:1]    # W_b as per-partition bias
            mbT = pkf_s[:, 1:2]    # M_b as per-partition bias
            onesT = pkf_s[:, 2:3]  # ones column (f32) for partition reduce
            khb = pkf_s[:, 3:4]    # -D/2

            # warm-up: a tiny gather primes the SWDGE path while the real
            # index tensors are still loading
            warmidx = const.tile([P, 1], I16)
            nc.vector.memset(warmidx[:], 0)
            warmout = const.tile([P, 1, E], F8)
            nc.gpsimd.dma_gather(warmout[:], emb[0:128, :], warmidx[:],
                                 16, 16, E, queue_num=0)

            # dummy ln(1) so the natural_log act table (which contains
            # every function used: relu/identity/square/copy/ln) loads once
            # during startup instead of mid-pipeline
            actwarm = const.tile([1, 1], F32)
            nc.scalar.activation(actwarm[0:1, :], pkf_s[0:1, 2:3], Ln)

            # ---- persistent intermediates ----
            stg = pers.tile([P, ESLOTS // P, E], F8)        # fp8 staging
            pstg = pers.tile([P, PSLOTS // P, 2 * D], F16)  # 5KB/part
            relu_c = pers.tile([P, Bs * C], BF16)
            h1 = pers.tile([P, Bs], BF16)
            h2 = pers.tile([P, Bs], BF16)
            z_s = pers.tile([P, Bs], F32)
            q_s = pers.tile([P, Bs], F32)
            s0f = pers.tile([P, Bs], F32)
            t1_s = pers.tile([P, Bs], F32)
            lnr_s = pers.tile([P, Bs], F32)
            klo_s = pers.tile([1, Bs], F32)

            # ---- stage 1: bucket-compacted gathers into SBUF staging ----
            # Windows are host-padded to their full static size (pad idx 0
            # rewrites bucket row 0 into unused slots), so every count is a
            # compile-time constant: no value_loads, no cnt registers.
            wb_engines = [nc.sync, nc.scalar]
            wb_i = [0]

            def s1_windows(dst, hbm, tab, o16, base, cap, elem):
                for w0 in range(0, cap, S1W):
                    n = min(S1W, cap - w0)
                    sl = dst[:, (base + w0) // P:(base + w0 + n) // P, :]
                    nc.gpsimd.dma_gather(
                        sl, tab,
                        pk16_s[:, o16 + (base + w0) // 16:
                               o16 + (base + w0 + n) // 16],
                        n, n, elem, queue_num=nextq(),
                    )
                # one bucket-granular writeback, alternating HWDGE engines:
                # staged row (base + j*128 + p) <- dst[p, base/128 + j, :]
                eng = wb_engines[wb_i[0] % 2]
                wb_i[0] += 1
                eng.dma_start(
                    out=hbm[base:base + cap, :].rearrange(
                        "(j p) e -> p j e", p=P),
                    in_=dst[:, base // P:(base + cap) // P, :],
                )

            for k in range(NBK):
                vhi = min(V, BK * (k + 1))
                s1_windows(stg, staged, emb[BK * k: vhi, :], O_SIDX,
                           EBASE[k], ECAPS[k], E)
            for k in range(NBK):
                vhi = min(V, BK * (k + 1))
                s1_windows(pstg, staged_pr, pcat[BK * k: vhi, :], O_PIDX,
                           PBASE[k], PCAPS[k], 2 * D)

            # ---- stage 2: SBUF-source transpose regathers ----
            # slot id i = rank*128 + partition with tokens_per_rank=128, so
            # the stage-2 index IS the staged slot id. One 512-idx call per
            # destination window tile (ucode transpose-mode limit).
            def sgather(out_tile, src, col0, sbuf=False):
                if sbuf:
                    nc.gpsimd.dma_gather(
                        out_tile[:], src[:],
                        pk16_s[:, col0:col0 + S2W // 16],
                        S2W, S2W, E, transpose=True,
                        queue_num=nextq(),
                        sbuf_tokens_per_rank=P,
                        sbuf_free_dim_per_rank=512,
                    )
                else:
                    nc.gpsimd.dma_gather(
                        out_tile[:], src[:, :],
                        pk16_s[:, col0:col0 + S2W // 16],
                        S2W, S2W, E, transpose=True,
                        queue_num=nextq(),
                    )

            priw = []
            for hf in range(2):
                pw = pers.tile([P, 2, S2W], F16, tag=f"priw{hf}")
                nc.gpsimd.dma_gather(
                    pw[:], pstg[:],
                    pk16_s[:, O_RPIDX + hf * S2W // 16:
                           O_RPIDX + (hf + 1) * S2W // 16],
                    S2W, S2W, 2 * D, transpose=True, queue_num=nextq(),
                    sbuf_tokens_per_rank=P, sbuf_free_dim_per_rank=512,
                )
                priw.append(pw)

            # x projection: h1 = relu(M_w @ emb_xT + M_b)
            def proj_pair(pp, wt):
                # fp8 transpose-gather output is u16-granular: partition k
                # holds emb dims (2k, 2k+1) as the byte pair of each token.
                # Contract even dims (lhsT mw8[:, :D] = M_w.T[0::2]) and odd
                # dims via stride-2 byte views of the same tile.
                v = wt[:].rearrange("p a j -> p (a j)").rearrange(
                    "p (j two) -> p j two", two=2)
                for kk in range(2):
                    nc.tensor.matmul(
                        pp[:], lhsT=mw8_s[:, kk * D:(kk + 1) * D],
                        rhs=v[:, :, kk],
                        start=(kk == 0), stop=(kk == 1),
                    )

            for w in range(Bs // S2W):
                xw = wtp.tile([P, 2, S2W], F8, tag="wt")
                nc.gpsimd.dma_gather(
                    xw[:], stg[:],
                    pk16_s[:, O_XIDX + w * S2W // 16:
                           O_XIDX + (w + 1) * S2W // 16],
                    S2W, S2W, E, transpose=True, queue_num=nextq(),
                    sbuf_tokens_per_rank=P, sbuf_free_dim_per_rank=256,
                )
                pp = psp.tile([P, 512], F32, tag="pp")
                proj_pair(pp, xw)
                nc.scalar.activation(h1[:, 512 * w:512 * (w + 1)], pp[:],
                                     Relu, bias=mbT)

            def kl_quarter(q):
                qs = slice(256 * q, 256 * (q + 1))
                pq = slice(256 * (q % 2), 256 * (q % 2 + 1))
                pw = priw[q // 2]
                pu = psm.tile([P, 256], F32, tag="ms")
                nc.tensor.matmul(pu[:], lhsT=uwt_s[:, 0:D], rhs=h1[:, qs],
                                 start=True, stop=False)
                nc.tensor.matmul(pu[:], lhsT=uwt_s[:, D:2 * D], rhs=h2[:, qs],
                                 start=False, stop=False)
                nc.tensor.matmul(pu[:], lhsT=ident_s[:], rhs=pw[:, 0, pq],
                                 start=False, stop=True)
                pz = psm.tile([P, 256], F32, tag="ms")
                nc.tensor.matmul(pz[:], lhsT=wwt_s[:, 0:D], rhs=h1[:, qs],
                                 start=True, stop=False)
                nc.tensor.matmul(pz[:], lhsT=wwt_s[:, D:2 * D], rhs=h2[:, qs],
                                 start=False, stop=True)
                # ACT: z (with W_b bias), q = (mu-m0)^2, s0 -> f32
                nc.scalar.activation(z_s[:, qs], pz[:], Identity, bias=wbT)
                nc.scalar.square(q_s[:, qs], pu[:])
                nc.scalar.copy(s0f[:, qs], pw[:, 1, pq])
                # rs = 1/softplus(z) = RC0 + RC1*z + RC2*z^2
                nc.vector.tensor_scalar(t1_s[:, qs], z_s[:, qs], RC2, RC1,
                                        TS.mult, TS.add)
                nc.vector.tensor_mul(t1_s[:, qs], t1_s[:, qs], z_s[:, qs])
                nc.vector.tensor_scalar_add(t1_s[:, qs], t1_s[:, qs], RC0)
                # w = q + s0 (in q); r = s0*rs (in s0f); wr = w*rs (in q)
                nc.vector.tensor_add(q_s[:, qs], q_s[:, qs], s0f[:, qs])
                nc.vector.tensor_mul(s0f[:, qs], s0f[:, qs], t1_s[:, qs])
                nc.scalar.activation(lnr_s[:, qs], s0f[:, qs], Ln)
                nc.vector.tensor_mul(q_s[:, qs], q_s[:, qs], t1_s[:, qs])
                nc.vector.tensor_sub(q_s[:, qs], q_s[:, qs], lnr_s[:, qs])
                # kl = 0.5 * sum_d(q) - D/2 via ones-matmul + ACT scale/bias
                pr = psr.tile([1, 256], F32, tag="pr")
                nc.tensor.matmul(pr[:], lhsT=onesT, rhs=q_s[:, qs],
                                 start=True, stop=True)
                nc.scalar.activation(klo_s[0:1, qs], pr[:], Identity,
                                     bias=khb[0:1, :], scale=0.5)

            # ---- ctx chunks: gather -> project -> relu -> C-sum ----
            for ch in range(NCH):
                t0 = ch * TPC
                for w in range(TPC // S2W):
                    wt = wtp.tile([P, 2, S2W], F8, tag="wt")
                    sgather(wt, staged, O_RIDX + (t0 + w * S2W) // 16)
                    pp = psp.tile([P, 512], F32, tag="pp")
                    proj_pair(pp, wt)
                    nc.scalar.activation(
                        relu_c[:, t0 + 512 * w:t0 + 512 * (w + 1)], pp[:],
                        Relu, bias=mbT)
                nb = TPC // C
                with nc.allow_low_precision(
                        reason="10-wide bf16 sum of O(0.01) relu values"):
                    nc.vector.tensor_reduce(
                        out=h2[:, ch * nb:(ch + 1) * nb],
                        in_=relu_c[:, t0:t0 + TPC].rearrange(
                            "p (b c) -> p b c", c=C),
                        axis=AX, op=TS.add,
                    )
                if ch >= 1:
                    kl_quarter(ch - 1)
            kl_quarter(3)

            nc.sync.dma_start(out=klo[:], in_=klo_s[:])

    # Spread SWDGE work over the 4 queues: queue = DMASW sem lane % 4, so each
    # of the 8 Tile DMA-SW lanes is serviced by exactly one queue.
    import re
    for inst in nc.inst_map.values():
        if isinstance(inst, mybir.InstDMAGatherAnt):
            si = inst.sync_info
            m = re.match(r"DMASW(\d+)_", si.on_update[0].ant_name)
            if m:
                inst.queue_num = int(m.group(1)) % 4

    nc.compile()
    return nc


def _pack_idx16(flat, pad_to):
    """dma_gather idx layout: [128, n/16] int16; entry i at [i%16, i//16],
    replicated across the 8 Q7 core partition groups."""
    t = np.full(pad_to, -1, np.int16)
    t[:len(flat)] = flat
    block = t.reshape(pad_to // 16, 16).T       # [16, n/16]
    return np.ascontiguousarray(np.tile(block, (8, 1)))


def _prep_core(xs, cs):
    """Build stage-1/2 index tensors for one core's shard."""
    toks = np.concatenate([cs.reshape(-1), xs]).astype(np.int64)  # ctx then x
    bkt = toks // BK
    # unused slot-range tail stays idx 0: windows run at full static count
    # (pad gathers rewrite bucket row 0 into unused slots, harmlessly)
    sidx_flat = np.zeros(ESLOTS, np.int16)
    slot = np.empty(NTOK, np.int64)
    for k in range(NBK):
        sel = np.flatnonzero(bkt == k)
        uniq, inv = np.unique(toks[sel] - BK * k, return_inverse=True)
        n = uniq.size
        assert n <= ECAPS[k], (k, n)
        sidx_flat[EBASE[k]:EBASE[k] + n] = uniq.astype(np.int16)
        slot[sel] = EBASE[k] + inv
    # priors (x tokens only)
    xb = xs // BK
    pidx_flat = np.zeros(PSLOTS, np.int16)
    pslot = np.empty(Bs, np.int64)
    for k in range(NBK):
        sel = np.flatnonzero(xb == k)
        n = sel.size
        assert n <= PCAPS[k], (k, n)
        pidx_flat[PBASE[k]:PBASE[k] + n] = (xs[sel] - BK * k).astype(np.int16)
        pslot[sel] = PBASE[k] + np.arange(n)
    pk16 = np.concatenate([
        _pack_idx16(sidx_flat, ESLOTS),
        _pack_idx16(slot[:Bs * C].astype(np.int16), Bs * C),
        _pack_idx16(slot[Bs * C:].astype(np.int16), Bs),
        _pack_idx16(pidx_flat, PSLOTS),
        _pack_idx16(pslot.astype(np.int16), Bs),
    ], axis=1)
    return {"pk16": np.ascontiguousarray(pk16)}


def kernel(x, context, W_emb, M_w, M_b, U_w, U_b, W_w, W_b, prior_mus,
           prior_sigmas):
    global last_results
    if "nc" not in _CACHE:
        _CACHE["nc"] = _build_nc()
    nc = _CACHE["nc"]

    x = np.asarray(x).astype(np.int64)
    context = np.asarray(context).astype(np.int64)
    W_emb = np.asarray(W_emb, dtype=np.float32)
    M_w = np.asarray(M_w, dtype=np.float32)
    M_b = np.asarray(M_b, dtype=np.float32)
    U_w = np.asarray(U_w, dtype=np.float32)
    U_b = np.asarray(U_b, dtype=np.float32)
    W_w = np.asarray(W_w, dtype=np.float32)
    W_b = np.asarray(W_b, dtype=np.float32)
    prior_mus = np.asarray(prior_mus, dtype=np.float32)
    prior_sigmas = np.asarray(prior_sigmas, dtype=np.float32)

    emb_8 = np.ascontiguousarray(W_emb.astype(ml_dtypes.float8_e4m3fn))
    # fp16 prior table rows: [U_b - m0 | s0]  (negated m0' accumulates into
    # the mu PSUM via an identity matmul: pu = U@h + U_b - m0)
    pcat_h = np.ascontiguousarray(np.concatenate(
        [U_b[None, :] - prior_mus, prior_sigmas],
        axis=1).astype(np.float16))
    MwT = M_w.T  # [E, D]
    # even/odd E-rows: fp8 transpose gather puts dims (2k, 2k+1) on part. k
    mw8_h = np.ascontiguousarray(np.concatenate(
        [MwT[0::2, :], MwT[1::2, :]], axis=1)).astype(ml_dtypes.float8_e4m3fn)
    scale = np.ones((2 * D,), np.float32)
    scale[:D] = float(C)     # C-fold of the repeated relu(Rw) half of h
    UT = (U_w * scale[None, :]).T
    WT = (W_w * scale[None, :]).T
    uwt_h = np.concatenate([UT[0:D], UT[D:2 * D]], axis=1)
    wwt_h = np.concatenate([WT[0:D], WT[D:2 * D]], axis=1)
    pkw_h = np.ascontiguousarray(
        np.concatenate([uwt_h, wwt_h], axis=1)).astype(ml_dtypes.bfloat16)
    pkh_h = np.ascontiguousarray(np.eye(P, dtype=np.float16))
    pkf_h = np.zeros((P, 4), np.float32)
    pkf_h[:, 0] = W_b
    pkf_h[:, 1] = M_b
    pkf_h[:, 2] = 1.0
    pkf_h[:, 3] = -float(D) / 2.0
    pkf_h = np.ascontiguousarray(pkf_h)

    in_maps = []
    for c in range(NCORES):
        m = _prep_core(x[c * Bs:(c + 1) * Bs], context[c * Bs:(c + 1) * Bs])
        m.update({
            "emb": emb_8, "pcat": pcat_h,
            "pkw": pkw_h, "pk8": mw8_h, "pkh": pkh_h, "pkf": pkf_h,
        })
        in_maps.append(m)

    res = run_bass_kernel_spmd(nc, in_maps, core_ids=list(range(NCORES)))
    last_results = res

    out = np.empty((B,), np.float32)
    for c in range(NCORES):
        out[c * Bs:(c + 1) * Bs] = res.results[c]["klo"].reshape(-1)
    return out


# revision 23
# speedup vs baseline: 1.0355x; 1.0355x over previous
"""Bass/Trainium2 kernel for nn_BayesianSkipgram (KL loss over skip-gram posterior).

Strategy (8 NeuronCores, data-parallel over batch; Bs=1024 per core):
  - Two-level gather, fully on-chip staging:
      stage 1: 4 bucket-compacted gathers (int16 local ids per 32767-row
               vocab bucket) land embedding rows for ALL 11264 token
               instances (ctx+x, no dedup) in an SBUF staging tile;
               4 more calls stage the fp16 prior rows [U_b-m0 | s0].
      stage 2: SBUF-source transpose-mode dma_gather (idx = staged slot id
               with tokens_per_rank=128) lands [E, token] tiles directly in
               original (b, c) order -- no HBM staging round trip, no PE
               transposes. 6 calls total (x, priors, 4 ctx chunks).
  - Projection RcT[D, tok] = M_w @ embT via PE (bf16) per 512-col PSUM bank,
    relu+bias via ACT, context sum via strided free-axis reduce.
  - KL computed in TRANSPOSED orientation [D, b]: mu/z via 2 matmuls each
    (uwt/wwt halves vs h1/h2), -(m0 - U_b) accumulated into the mu PSUM via
    an identity-f16 matmul, W_b folded in as an ACT bias.
  - 1/sigma = 1/softplus(z) as a degree-2 polynomial in z (|z| < 0.07 at
    this model scale; 8e-5 rel err over 2x the range); both log terms come
    from one ACT ln: ln sigma - ln s0 = -ln(s0/sigma) = -ln(s0 * rs).
  - Final sum over D via a ones-vector fp32 matmul (partition reduce on PE),
    kl = 0.5*sum - D/2 via ACT scale+bias; output is [1, Bs] f32.
  - Work is pipelined per ctx chunk: KL for batch half 0 runs while chunks
    2-3 are still gathering.
Host work is sharding/layout only: dtype casts, bucket sorting and index
packing, weight transposition, output reassembly.
"""

import numpy as np
import ml_dtypes

import concourse.bass as bass
import concourse.mybir as mybir
from concourse import bacc
from concourse import tile
from concourse.bass_utils import run_bass_kernel_spmd
from concourse.library_config import mlp

# Problem constants (hardcoded per harness contract)
V, E, D, B, C = 100000, 256, 128, 8192, 10
NCORES = 8
Bs = B // NCORES            # 1024 batch items per core
P = 128
NTOK = Bs * C + Bs          # 11264 gathered token instances (ctx then x)
BK = 32767                  # int16 vocab bucket size
NBK = 4
ECAPS = (3712, 3712, 3712, 384)      # emb stage-1 per-bucket caps (dedup'd)
EBASE = (0, 3712, 7424, 11136)
ESLOTS = sum(ECAPS)                  # 11520 staging slots
PCAPS = (384, 384, 384, 128)         # prior stage-1 per-bucket caps
PBASE = (0, 384, 768, 1152)
PSLOTS = sum(PCAPS)                  # 1280
S1W = 1024                           # stage-1 window cap (ucode limit)
S2W = 512                            # transpose-mode window cap (ucode limit)
NCH = 4
TPC = (Bs * C) // NCH                # 2560 ctx tokens per stage-2 chunk
HB = Bs // 2                         # 512-wide KL half

# 1/softplus(z) ~= RC0 + RC1*z + RC2*z^2 (fit on |z| <= 0.125)
RC0, RC1, RC2 = 1.44268652, -1.04204494, 0.49387287

F32 = mybir.dt.float32
BF16 = mybir.dt.bfloat16
F16 = mybir.dt.float16
F8 = mybir.dt.float8e4
I32 = mybir.dt.int32
I16 = mybir.dt.int16

_CACHE = {}
last_results = None  # set by kernel(); test.py reads exec_time_ns from here


def _build_nc():
    nc = bacc.Bacc(
        "TRN2",
        target_bir_lowering=False,
        debug=False,
        num_devices=NCORES,
        num_swdge_queues=4,
    )

    emb = nc.dram_tensor("emb", [V, E], F8, kind="ExternalInput")
    pcat = nc.dram_tensor("pcat", [V, 2 * D], F16, kind="ExternalInput")
    pk16 = nc.dram_tensor("pk16", [P, ESLOTS // 16 + Bs * C // 16 + Bs // 16
                                   + PSLOTS // 16 + Bs // 16], I16,
                          kind="ExternalInput")
    pkw = nc.dram_tensor("pkw", [P, 2 * 2 * D], BF16, kind="ExternalInput")
    pk8 = nc.dram_tensor("pk8", [P, 2 * D], F8, kind="ExternalInput")
    pkh = nc.dram_tensor("pkh", [P, P], F16, kind="ExternalInput")
    pkf = nc.dram_tensor("pkf", [P, 4], F32, kind="ExternalInput")
    klo = nc.dram_tensor("klo", [1, Bs], F32, kind="ExternalOutput")
    # HBM staging (ExternalOutput => contiguous runtime-allocated tensors;
    # Internal DRAM scratch may be paged, breaking flat base+idx*stride)
    staged = nc.dram_tensor("staged", [ESLOTS, E], F8, kind="ExternalOutput")
    staged_pr = nc.dram_tensor("staged_pr", [PSLOTS, 2 * D], F16,
                               kind="ExternalOutput")

    Relu = mybir.ActivationFunctionType.Relu
    Identity = mybir.ActivationFunctionType.Identity
    Ln = mybir.ActivationFunctionType.Ln
    TS = mybir.AluOpType
    AX = mybir.AxisListType.X

    # pk16 column offsets (int16 units)
    O_SIDX = 0
    O_RIDX = O_SIDX + ESLOTS // 16           # ctx stage-2 slots
    O_XIDX = O_RIDX + Bs * C // 16           # x stage-2 slots
    O_PIDX = O_XIDX + Bs // 16               # prior stage-1 local ids
    O_RPIDX = O_PIDX + PSLOTS // 16          # prior stage-2 slots

    def nextq():
        # placeholder; real queue assignment happens post-schedule, derived
        # from the Tile-assigned DMASW sem lane (one lane must map to exactly
        # one SWDGE queue)
        return 0

    with tile.TileContext(nc) as tc:
        with (
            tc.tile_pool(name="const", bufs=1) as const,
            tc.tile_pool(name="pers", bufs=1) as pers,
            tc.tile_pool(name="wtp", bufs=8) as wtp,
            tc.tile_pool(name="psp", bufs=3, space="PSUM") as psp,
            tc.tile_pool(name="psm", bufs=2, space="PSUM") as psm,
            tc.tile_pool(name="psr", bufs=2, space="PSUM") as psr,
        ):
            nc.gpsimd.load_library(mlp)

            # ---- constants into SBUF (5 DMAs) ----
            pk16_s = const.tile([P, pk16.shape[1]], I16)
            nc.sync.dma_start(out=pk16_s[:], in_=pk16[:])
            pkw_s = const.tile([P, 2 * 2 * D], BF16)
            nc.sync.dma_start(out=pkw_s[:], in_=pkw[:])
            mw8_s = const.tile([P, 2 * D], F8)
            nc.sync.dma_start(out=mw8_s[:], in_=pk8[:])
            ident_s = const.tile([P, P], F16)
            nc.sync.dma_start(out=ident_s[:], in_=pkh[:])
            pkf_s = const.tile([P, 4], F32)
            nc.sync.dma_start(out=pkf_s[:], in_=pkf[:])

            uwt_s = pkw_s[:, 0:2 * D]
            wwt_s = pkw_s[:, 2 * D:4 * D]
            wbT = pkf_s[:, 0:1]    # W_b as per-partition bias
            mbT = pkf_s[:, 1:2]    # M_b as per-partition bias
            onesT = pkf_s[:, 2:3]  # ones column (f32) for partition reduce
            khb = pkf_s[:, 3:4]    # -D/2

            # warm-up: a tiny gather primes the SWDGE path while the real
            # index tensors are still loading
            warmidx = const.tile([P, 1], I16)
            nc.vector.memset(warmidx[:], 0)
            warmout = const.tile([P, 1, E], F8)
            nc.gpsimd.dma_gather(warmout[:], emb[0:128, :], warmidx[:],
                                 16, 16, E, queue_num=0)

            # dummy ln(1) so the natural_log act table (which contains
            # every function used: relu/identity/square/copy/ln) loads once
            # during startup instead of mid-pipeline
            actwarm = const.tile([1, 1], F32)
            nc.scalar.activation(actwarm[0:1, :], pkf_s[0:1, 2:3], Ln)

            # ---- persistent intermediates ----
            stg = pers.tile([P, ESLOTS // P, E], F8)        # fp8 staging
            pstg = pers.tile([P, PSLOTS // P, 2 * D], F16)  # 5KB/part
            relu_c = pers.tile([P, Bs * C], BF16)
            h1 = pers.tile([P, Bs], BF16)
            h2 = pers.tile([P, Bs], BF16)
            z_s = pers.tile([P, Bs], F32)
            q_s = pers.tile([P, Bs], F32)
            s0f = pers.tile([P, Bs], F32)
            t1_s = pers.tile([P, Bs], F32)
            lnr_s = pers.tile([P, Bs], F32)
            klo_s = pers.tile([1, Bs], F32)

            # ---- stage 1: bucket-compacted gathers into SBUF staging ----
            # Windows are host-padded to their full static size (pad idx 0
            # rewrites bucket row 0 into unused slots), so every count is a
            # compile-time constant: no value_loads, no cnt registers.
            wb_engines = [nc.sync, nc.scalar]
            wb_i = [0]

            def s1_windows(dst, hbm, tab, o16, base, cap, elem):
                for w0 in range(0, cap, S1W):
                    n = min(S1W, cap - w0)
                    sl = dst[:, (base + w0) // P:(base + w0 + n) // P, :]
                    nc.gpsimd.dma_gather(
                        sl, tab,
                        pk16_s[:, o16 + (base + w0) // 16:
                               o16 + (base + w0 + n) // 16],
                        n, n, elem, queue_num=nextq(),
                    )
                    # per-window writeback, alternating HWDGE engines:
                    # staged row (base+w0+j*128+p) <- sl[p, j, :]
                    eng = wb_engines[wb_i[0] % 2]
                    wb_i[0] += 1
                    eng.dma_start(
                        out=hbm[base + w0:base + w0 + n, :].rearrange(
                            "(j p) e -> p j e", p=P),
                        in_=sl,
                    )

            for k in range(NBK):
                vhi = min(V, BK * (k + 1))
                s1_windows(stg, staged, emb[BK * k: vhi, :], O_SIDX,
                           EBASE[k], ECAPS[k], E)
            for k in range(NBK):
                vhi = min(V, BK * (k + 1))
                s1_windows(pstg, staged_pr, pcat[BK * k: vhi, :], O_PIDX,
                           PBASE[k], PCAPS[k], 2 * D)

            # ---- stage 2: SBUF-source transpose regathers ----
            # slot id i = rank*128 + partition with tokens_per_rank=128, so
            # the stage-2 index IS the staged slot id. One 512-idx call per
            # destination window tile (ucode transpose-mode limit).
            def sgather(out_tile, src, col0, sbuf=False):
                if sbuf:
                    nc.gpsimd.dma_gather(
                        out_tile[:], src[:],
                        pk16_s[:, col0:col0 + S2W // 16],
                        S2W, S2W, E, transpose=True,
                        queue_num=nextq(),
                        sbuf_tokens_per_rank=P,
                        sbuf_free_dim_per_rank=512,
                    )
                else:
                    nc.gpsimd.dma_gather(
                        out_tile[:], src[:, :],
                        pk16_s[:, col0:col0 + S2W // 16],
                        S2W, S2W, E, transpose=True,
                        queue_num=nextq(),
                    )

            priw = []
            for hf in range(2):
                pw = pers.tile([P, 2, S2W], F16, tag=f"priw{hf}")
                sgather(pw, staged_pr, O_RPIDX + hf * S2W // 16)
                priw.append(pw)

            # x projection: h1 = relu(M_w @ emb_xT + M_b)
            def proj_pair(pp, wt):
                # fp8 transpose-gather output is u16-granular: partition k
                # holds emb dims (2k, 2k+1) as the byte pair of each token.
                # Contract even dims (lhsT mw8[:, :D] = M_w.T[0::2]) and odd
                # dims via stride-2 byte views of the same tile.
                v = wt[:].rearrange("p a j -> p (a j)").rearrange(
                    "p (j two) -> p j two", two=2)
                for kk in range(2):
                    nc.tensor.matmul(
                        pp[:], lhsT=mw8_s[:, kk * D:(kk + 1) * D],
                        rhs=v[:, :, kk],
                        start=(kk == 0), stop=(kk == 1),
                    )

            for w in range(Bs // S2W):
                xw = wtp.tile([P, 2, S2W], F8, tag="wt")
                sgather(xw, staged, O_XIDX + w * S2W // 16)
                pp = psp.tile([P, 512], F32, tag="pp")
                proj_pair(pp, xw)
                nc.scalar.activation(h1[:, 512 * w:512 * (w + 1)], pp[:],
                                     Relu, bias=mbT)

            def kl_quarter(q):
                qs = slice(256 * q, 256 * (q + 1))
                pq = slice(256 * (q % 2), 256 * (q % 2 + 1))
                pw = priw[q // 2]
                pu = psm.tile([P, 256], F32, tag="ms")
                nc.tensor.matmul(pu[:], lhsT=uwt_s[:, 0:D], rhs=h1[:, qs],
                                 start=True, stop=False)
                nc.tensor.matmul(pu[:], lhsT=uwt_s[:, D:2 * D], rhs=h2[:, qs],
                                 start=False, stop=False)
                nc.tensor.matmul(pu[:], lhsT=ident_s[:], rhs=pw[:, 0, pq],
                                 start=False, stop=True)
                pz = psm.tile([P, 256], F32, tag="ms")
                nc.tensor.matmul(pz[:], lhsT=wwt_s[:, 0:D], rhs=h1[:, qs],
                                 start=True, stop=False)
                nc.tensor.matmul(pz[:], lhsT=wwt_s[:, D:2 * D], rhs=h2[:, qs],
                                 start=False, stop=True)
                # ACT: z (with W_b bias), q = (mu-m0)^2, s0 -> f32
                nc.scalar.activation(z_s[:, qs], pz[:], Identity, bias=wbT)
                nc.scalar.square(q_s[:, qs], pu[:])
                nc.scalar.copy(s0f[:, qs], pw[:, 1, pq])
                # rs = 1/softplus(z) = RC0 + RC1*z + RC2*z^2
                nc.vector.tensor_scalar(t1_s[:, qs], z_s[:, qs], RC2, RC1,
                                        TS.mult, TS.add)
                nc.vector.tensor_mul(t1_s[:, qs], t1_s[:, qs], z_s[:, qs])
                nc.vector.tensor_scalar_add(t1_s[:, qs], t1_s[:, qs], RC0)
                # w = q + s0 (in q); r = s0*rs (in s0f); wr = w*rs (in q)
                nc.vector.tensor_add(q_s[:, qs], q_s[:, qs], s0f[:, qs])
                nc.vector.tensor_mul(s0f[:, qs], s0f[:, qs], t1_s[:, qs])
                nc.scalar.activation(lnr_s[:, qs], s0f[:, qs], Ln)
                nc.vector.tensor_mul(q_s[:, qs], q_s[:, qs], t1_s[:, qs])
                nc.vector.tensor_sub(q_s[:, qs], q_s[:, qs], lnr_s[:, qs])
                # kl = 0.5 * sum_d(q) - D/2 via ones-matmul + ACT scale/bias
                pr = psr.tile([1, 256], F32, tag="pr")
                nc.tensor.matmul(pr[:], lhsT=onesT, rhs=q_s[:, qs],
                                 start=True, stop=True)
                nc.scalar.activation(klo_s[0:1, qs], pr[:], Identity,
                                     bias=khb[0:1, :], scale=0.5)

            # ---- ctx chunks: gather -> project -> relu -> C-sum ----
            for ch in range(NCH):
                t0 = ch * TPC
                for w in range(TPC // S2W):
                    wt = wtp.tile([P, 2, S2W], F8, tag="wt")
                    sgather(wt, staged, O_RIDX + (t0 + w * S2W) // 16)
                    pp = psp.tile([P, 512], F32, tag="pp")
                    proj_pair(pp, wt)
                    nc.scalar.activation(
                        relu_c[:, t0 + 512 * w:t0 + 512 * (w + 1)], pp[:],
                        Relu, bias=mbT)
                nb = TPC // C
                with nc.allow_low_precision(
                        reason="10-wide bf16 sum of O(0.01) relu values"):
                    nc.vector.tensor_reduce(
                        out=h2[:, ch * nb:(ch + 1) * nb],
                        in_=relu_c[:, t0:t0 + TPC].rearrange(
                            "p (b c) -> p b c", c=C),
                        axis=AX, op=TS.add,
                    )
                if ch >= 1:
                    kl_quarter(ch - 1)
            kl_quarter(3)

            nc.sync.dma_start(out=klo[:], in_=klo_s[:])

    # Spread SWDGE work over the 4 queues: queue = DMASW sem lane % 4, so each
    # of the 8 Tile DMA-SW lanes is serviced by exactly one queue.
    import re
    for inst in nc.inst_map.values():
        if isinstance(inst, mybir.InstDMAGatherAnt):
            si = inst.sync_info
            m = re.match(r"DMASW(\d+)_", si.on_update[0].ant_name)
            if m:
                inst.queue_num = int(m.group(1)) % 4

    nc.compile()
    return nc


def _pack_idx16(flat, pad_to):
    """dma_gather idx layout: [128, n/16] int16; entry i at [i%16, i//16],
    replicated across the 8 Q7 core partition groups."""
    t = np.full(pad_to, -1, np.int16)
    t[:len(flat)] = flat
    block = t.reshape(pad_to // 16, 16).T       # [16, n/16]
    return np.ascontiguousarray(np.tile(block, (8, 1)))


def _prep_core(xs, cs):
    """Build stage-1/2 index tensors for one core's shard."""
    toks = np.concatenate([cs.reshape(-1), xs]).astype(np.int64)  # ctx then x
    bkt = toks // BK
    # unused slot-range tail stays idx 0: windows run at full static count
    # (pad gathers rewrite bucket row 0 into unused slots, harmlessly)
    sidx_flat = np.zeros(ESLOTS, np.int16)
    slot = np.empty(NTOK, np.int64)
    for k in range(NBK):
        sel = np.flatnonzero(bkt == k)
        uniq, inv = np.unique(toks[sel] - BK * k, return_inverse=True)
        n = uniq.size
        assert n <= ECAPS[k], (k, n)
        sidx_flat[EBASE[k]:EBASE[k] + n] = uniq.astype(np.int16)
        slot[sel] = EBASE[k] + inv
    # priors (x tokens only)
    xb = xs // BK
    pidx_flat = np.zeros(PSLOTS, np.int16)
    pslot = np.empty(Bs, np.int64)
    for k in range(NBK):
        sel = np.flatnonzero(xb == k)
        n = sel.size
        assert n <= PCAPS[k], (k, n)
        pidx_flat[PBASE[k]:PBASE[k] + n] = (xs[sel] - BK * k).astype(np.int16)
        pslot[sel] = PBASE[k] + np.arange(n)
    pk16 = np.concatenate([
        _pack_idx16(sidx_flat, ESLOTS),
        _pack_idx16(slot[:Bs * C].astype(np.int16), Bs * C),
        _pack_idx16(slot[Bs * C:].astype(np.int16), Bs),
        _pack_idx16(pidx_flat, PSLOTS),
        _pack_idx16(pslot.astype(np.int16), Bs),
    ], axis=1)
    return {"pk16": np.ascontiguousarray(pk16)}


def kernel(x, context, W_emb, M_w, M_b, U_w, U_b, W_w, W_b, prior_mus,
           prior_sigmas):
    global last_results
    if "nc" not in _CACHE:
        _CACHE["nc"] = _build_nc()
    nc = _CACHE["nc"]

    x = np.asarray(x).astype(np.int64)
    context = np.asarray(context).astype(np.int64)
    W_emb = np.asarray(W_emb, dtype=np.float32)
    M_w = np.asarray(M_w, dtype=np.float32)
    M_b = np.asarray(M_b, dtype=np.float32)
    U_w = np.asarray(U_w, dtype=np.float32)
    U_b = np.asarray(U_b, dtype=np.float32)
    W_w = np.asarray(W_w, dtype=np.float32)
    W_b = np.asarray(W_b, dtype=np.float32)
    prior_mus = np.asarray(prior_mus, dtype=np.float32)
    prior_sigmas = np.asarray(prior_sigmas, dtype=np.float32)

    emb_8 = np.ascontiguousarray(W_emb.astype(ml_dtypes.float8_e4m3fn))
    # fp16 prior table rows: [U_b - m0 | s0]  (negated m0' accumulates into
    # the mu PSUM via an identity matmul: pu = U@h + U_b - m0)
    pcat_h = np.ascontiguousarray(np.concatenate(
        [U_b[None, :] - prior_mus, prior_sigmas],
        axis=1).astype(np.float16))
    MwT = M_w.T  # [E, D]
    # even/odd E-rows: fp8 transpose gather puts dims (2k, 2k+1) on part. k
    mw8_h = np.ascontiguousarray(np.concatenate(
        [MwT[0::2, :], MwT[1::2, :]], axis=1)).astype(ml_dtypes.float8_e4m3fn)
    scale = np.ones((2 * D,), np.float32)
    scale[:D] = float(C)     # C-fold of the repeated relu(Rw) half of h
    UT = (U_w * scale[None, :]).T
    WT = (W_w * scale[None, :]).T
    uwt_h = np.concatenate([UT[0:D], UT[D:2 * D]], axis=1)
    wwt_h = np.concatenate([WT[0:D], WT[D:2 * D]], axis=1)
    pkw_h = np.ascontiguousarray(
        np.concatenate([uwt_h, wwt_h], axis=1)).astype(ml_dtypes.bfloat16)
    pkh_h = np.ascontiguousarray(np.eye(P, dtype=np.float16))
    pkf_h = np.zeros((P, 4), np.float32)
    pkf_h[:, 0] = W_b
    pkf_h[:, 1] = M_b
    pkf_h[:, 2] = 1.0
    pkf_h[:, 3] = -float(D) / 2.0
    pkf_h = np.ascontiguousarray(pkf_h)

    in_maps = []
    for c in range(NCORES):
        m = _prep_core(x[c * Bs:(c + 1) * Bs], context[c * Bs:(c + 1) * Bs])
        m.update({
            "emb": emb_8, "pcat": pcat_h,
            "pkw": pkw_h, "pk8": mw8_h, "pkh": pkh_h, "pkf": pkf_h,
        })
        in_maps.append(m)

    res = run_bass_kernel_spmd(nc, in_maps, core_ids=list(range(NCORES)))
    last_results = res

    out = np.empty((B,), np.float32)
    for c in range(NCORES):
        out[c * Bs:(c + 1) * Bs] = res.results[c]["klo"].reshape(-1)
    return out
